# revision 1
# baseline (speedup 1.0000x reference)
"""DiscFace AM-softmax loss kernel for 8 TRN2 NeuronCores.

Strategy (tensor-parallel over classes):
  - id_agent/b sharded row-wise: core k owns classes [k*12500, (k+1)*12500),
    padded to 12800 rows with zeros (pad rows produce logits == 0 exactly,
    contributing exp(0) == 1 each to the softmax denominator; the constant
    8*300 = 2400 is subtracted during the final correction).
  - x replicated; each core computes partial logits x_n @ w_shard.T (bf16
    matmul, fp32 accumulate), and the softmax denominator partials via the
    ACT engine's fused exp+accumulate. No max subtraction is needed:
    logits are bounded by SCALE=64 and exp(64) fits comfortably in fp32.
  - The margin on the target logit is applied via a scalar correction:
    Z += exp(64*st - 22.4) - exp(64*st), with st = cos(x_n, w_target)
    computed exactly (fp32) from an on-device indirect-DMA gather of the
    owned target rows (ownership-masked, clamped local indices).
  - One AllReduce of a [128, 24] payload (Z partials / masked st / masked
    residual norms), then every core finishes the focal + disc loss math;
    core 0's [1] output is returned.
"""

import os
import sys

import numpy as np

sys.path.insert(0, "/opt/trn_rl_repo")

from concourse import bass, mybir, tile  # noqa: E402
from concourse.bass_utils import run_bass_kernel_spmd  # noqa: E402

B, D, C = 1024, 512, 100000
NCORES = 8
CPER = C // NCORES          # 12500 real classes per core
CSH = 12800                 # padded shard rows (100 tiles of 128)
NPAD_TOTAL = float(NCORES * (CSH - CPER))   # 2400 pad contributions to Z
CT = CSH // 128             # 100 class tiles per core
CHUNK_T = 4                 # class tiles per matmul chunk (512 classes)
NCHUNK = CT // CHUNK_T      # 25 chunks
# Variable transpose-group sizes: small groups first so the matmul pipeline
# ramps up ~15us into the kernel instead of waiting for a full 20-tile
# stage-1 sweep; big groups amortize transpose + exp overheads after that.
GROUPS = [8, 12, 20, 20, 20, 20]    # tiles per group (sums to CT)
assert sum(GROUPS) == CT
GSTART = [sum(GROUPS[:g]) for g in range(len(GROUPS))]


def pieces_for(nt):
    # (first chunk, width-in-512-chunks) per psum piece
    return {
        8: ((0, 2),),
        12: ((0, 2), (2, 1)),
        20: ((0, 2), (2, 2), (4, 1)),
    }[nt]


GPIECE = [len(pieces_for(nt)) for nt in GROUPS]
PBASE = [sum(GPIECE[:g]) for g in range(len(GROUPS))]
NPIECE = sum(GPIECE)                # Z partial columns per batch tile
N_SCALARQ_TILES = 40                # early ia loads go via the ACT hwdge q
BT = B // 128               # 8 batch tiles
NDB = D // 128              # 4 contraction blocks

SCALE = 64.0
MARGIN = 0.35
LAMBDA = 0.4
SM = SCALE * MARGIN         # 22.4
LOG_SCALE = float(np.log(SCALE))
LOG_BCLIP = float(np.log(0.05))

F32 = mybir.dt.float32
BF16 = mybir.dt.bfloat16
I32 = mybir.dt.int32
AF = mybir.ActivationFunctionType
ALU = mybir.AluOpType
AX = mybir.AxisListType


# Engine-executed compute instruction classes. The TRN2 TPB instruction
# encoding has exactly ONE semaphore-wait slot (NEURON_ISA_TPB_EVENTS), and
# walrus refuses to encode instructions carrying more ("Too many sync wait
# commands" / "ISA wrong length"). Tile's scheduler attaches as many waits
# as the dependency structure demands, so after scheduling we move every
# wait off compute instructions onto same-engine NoOps (one wait each),
# which the sequencer executes in order just like inline waits.
# Classes whose waits we must not touch (pre-encoded raw blobs).
_NO_SPLIT_CLASSES = ("InstISA", "InstCall")


def split_multi_waits(nc):
    n_nops = 0
    for f in nc.m.functions:
        for bb in f.blocks:
            new_insts = []
            for inst in bb.instructions:
                si = inst.sync_info
                cls = type(inst).__name__
                # Raw-ISA-encoded instructions (pre-packed 64B blobs,
                # exposed via .isa_opcode) can carry NO inline wait at all;
                # regular TPB instructions can carry exactly one.
                zero_wait = (
                    cls != "InstISA"
                    and (hasattr(inst, "isa_opcode") or cls == "InstDmaTransposeAnt")
                )
                keep = 0 if zero_wait else 1
                if (
                    si is not None
                    and len(si.on_wait) > keep
                    and cls not in _NO_SPLIT_CLASSES
                ):
                    split = si.on_wait[:-keep] if keep else list(si.on_wait)
                    for w in split:
                        nop = mybir.InstNoOp(
                            name=nc.get_next_instruction_name(),
                            sync_info=mybir.SyncInfo(on_wait=[w], on_update=[]),
                            bass_nofuse=True,
                            engine=inst.engine,
                        )
                        nc.inst_map[nop.name] = nop
                        new_insts.append(nop)
                        n_nops += 1
                    inst.sync_info = mybir.SyncInfo(
                        on_wait=list(si.on_wait[-keep:]) if keep else [],
                        on_update=list(si.on_update),
                    )
                new_insts.append(inst)
            bb.instructions = new_insts
    return n_nops


def build_bass():
    nc = bass.Bass(trn_type="TRN2", num_devices=NCORES)

    x_d = nc.declare_dram_parameter("x", [B, D], F32, isOutput=False)
    ia_d = nc.declare_dram_parameter("ia", [CSH, D], F32, isOutput=False)
    bsh_d = nc.declare_dram_parameter("bsh", [CSH, D], F32, isOutput=False)
    toff_d = nc.declare_dram_parameter("toff", [128, BT], I32, isOutput=False)
    tmask_d = nc.declare_dram_parameter("tmask", [128, BT], F32, isOutput=False)
    out_d = nc.declare_dram_parameter("out", [1], F32, isOutput=True)

    ccin1 = nc.dram_tensor("ccin1", [128, 16], F32)
    ccout1 = nc.dram_tensor("ccout1", [128, 16], F32, addr_space="Shared")
    ccin2 = nc.dram_tensor("ccin2", [128, BT], F32)
    ccout2 = nc.dram_tensor("ccout2", [128, BT], F32, addr_space="Shared")
    tsc = nc.dram_tensor("tsc", [CSH, D], BF16)     # scaled bf16 staging

    # Register const APs for the nonzero activation biases we use.
    for v in (LOG_SCALE, LOG_BCLIP, -SM):
        t = nc.alloc_sbuf_tensor(f"const-f32-{v}", [128, 1], F32)
        nc.gpsimd.memset(t.ap(), v)
        nc.const_aps.aps[(F32, v)] = t.ap()
    nc.all_engine_barrier()

    with tile.TileContext(nc) as tc:
        with (
            tc.tile_pool(name="persist", bufs=1) as pp,
            tc.tile_pool(name="ia", bufs=6) as ia_pool,
            tc.tile_pool(name="scaled", bufs=6) as sc_pool,
            tc.tile_pool(name="idt", bufs=3) as idt_pool,
            tc.tile_pool(name="dump", bufs=3) as dump_pool,
            tc.tile_pool(name="gath", bufs=1) as g_pool,
            tc.tile_pool(name="work", bufs=3) as w_pool,
            tc.tile_pool(name="small", bufs=2) as s_pool,
            tc.tile_pool(name="psum", bufs=4, space="PSUM") as ps_pool,
        ):
            # ---------------- persistent tiles ----------------
            xn3 = pp.tile([128, BT, D], F32, tag="xn3")          # normalized x
            xT = pp.tile([128, NDB, B], BF16, tag="xT")          # [d, b] bf16
            ssx = pp.tile([128, BT], F32, tag="ssx")
            xscale = pp.tile([128, BT], F32, tag="xscale")
            ss2d = pp.tile([128, CT], F32, tag="ss2d")           # row sumsq
            scale2d = pp.tile([128, CT], F32, tag="scale2d")     # 64/norm
            zp2d = pp.tile([128, BT * NPIECE], F32, tag="zp2d")  # exp partials
            payload1 = pp.tile([128, 16], F32, tag="payload1")
            payload2 = pp.tile([128, BT], F32, tag="payload2")
            allred1 = pp.tile([128, 16], F32, tag="allred1")
            allred2 = pp.tile([128, BT], F32, tag="allred2")
            toffs = pp.tile([128, BT], I32, tag="toffs")
            tmasks = pp.tile([128, BT], F32, tag="tmasks")
            ones = pp.tile([128, 1], F32, tag="ones")
            ident = pp.tile([128, 128], F32, tag="ident")
            # disc-path persistents
            ng2 = pp.tile([128, BT], F32, tag="ng2")
            dot8 = pp.tile([128, BT], F32, tag="dot8")
            btn2 = pp.tile([128, BT], F32, tag="btn2")
            rn2 = pp.tile([128, BT], F32, tag="rn2")
            s1_8 = pp.tile([128, BT], F32, tag="s1_8")
            f8 = pp.tile([128, BT], F32, tag="f8")
            lb8 = pp.tile([128, BT], F32, tag="lb8")
            g3 = pp.tile([128, BT, D], F32, tag="g3")            # gathered ia rows
            btg3 = pp.tile([128, BT, D], F32, tag="btg3")        # gathered b rows

            nc.vector.memset(ones[:], 1.0)
            from concourse.masks import make_identity
            make_identity(nc, ident[:])

            # ---------------- phase 0: x normalize + transpose ----------------
            nc.gpsimd.dma_start(out=toffs[:], in_=toff_d[:])
            nc.gpsimd.dma_start(out=tmasks[:], in_=tmask_d[:])

            for bt in range(BT):
                nc.scalar.dma_start(
                    out=xn3[:, bt, :], in_=x_d[bt * 128:(bt + 1) * 128, :]
                )
                dmp = dump_pool.tile([128, D], F32, tag="dmpf32")
                nc.vector.scalar_tensor_tensor(
                    out=dmp[:], in0=xn3[:, bt, :], scalar=1.0,
                    in1=xn3[:, bt, :], op0=ALU.mult, op1=ALU.mult,
                    accum_out=ssx[:, bt:bt + 1],
                )
            # xscale = exp(-0.5 * log(ssx)) = 1/||x||
            nc.vector.tensor_scalar_max(out=ssx[:], in0=ssx[:], scalar1=1e-30)
            nc.scalar.activation(xscale[:], ssx[:], AF.Ln)
            nc.scalar.activation(xscale[:], xscale[:], AF.Exp, scale=-0.5)
            for bt in range(BT):
                nc.vector.tensor_scalar_mul(
                    out=xn3[:, bt, :], in0=xn3[:, bt, :],
                    scalar1=xscale[:, bt:bt + 1],
                )
                for db in range(NDB):
                    tp = ps_pool.tile([128, 1024], F32, tag="ps")
                    nc.tensor.transpose(
                        out=tp[:, 0:128],
                        in_=xn3[:, bt, db * 128:(db + 1) * 128],
                        identity=ident[:],
                    )
                    nc.vector.tensor_copy(
                        out=xT[:, db, bt * 128:(bt + 1) * 128],
                        in_=tp[:, 0:128],
                    )

            # ---------------- main class loop ----------------
            # Software-pipelined over transpose groups: stage 1 of group g+1
            # (load raw rows -> fused sumsq -> scale=64/||row|| via Ln/Exp ->
            # scale+cast bf16 -> 4-wide stage to DRAM scratch) is emitted
            # interleaved with group g's matmul/exp sweep so the PE never
            # stalls at group boundaries (keeps HAM at full clock). The
            # transpose is one big DRAM->SBUF xbar transfer per 128-d block.

            idts = {}

            def produce(g):
                """Generator emitting the full stage-1 pipeline for group g
                in interleavable 4-tile bundles (load+sumsq, batched scale,
                cast, 4-wide stage), then the group's DRAM-path
                transpose-loads. idts[g] is set by the final bundle."""
                nt = GROUPS[g]
                nq = nt // 4
                for q in range(nq):
                    i0 = GSTART[g] + q * 4
                    ia_ts = []
                    for j in range(4):
                        i = i0 + j
                        ia_t = ia_pool.tile([128, D], F32, tag="ia")
                        ia_ts.append(ia_t)
                        eng = nc.scalar if i < N_SCALARQ_TILES else nc.gpsimd
                        eng.dma_start(
                            out=ia_t[:], in_=ia_d[i * 128:(i + 1) * 128, :]
                        )
                        dmp = dump_pool.tile([128, D], F32, tag="dmpf32")
                        nc.vector.scalar_tensor_tensor(
                            out=dmp[:], in0=ia_t[:], scalar=1.0,
                            in1=ia_t[:], op0=ALU.mult, op1=ALU.mult,
                            accum_out=ss2d[:, i:i + 1],
                        )
                    # scale = exp(-0.5*ln(max(ss,eps)) + ln64) for these 4
                    lbuf = s_pool.tile([128, 4], F32, tag="lbuf")
                    nc.vector.tensor_scalar_max(
                        out=lbuf[:], in0=ss2d[:, i0:i0 + 4], scalar1=1e-30,
                    )
                    nc.scalar.activation(lbuf[:], lbuf[:], AF.Ln)
                    nc.scalar.activation(
                        scale2d[:, i0:i0 + 4], lbuf[:], AF.Exp,
                        scale=-0.5, bias=LOG_SCALE,
                    )
                    sc4 = sc_pool.tile([128, 4, D], BF16, tag="scaled")
                    for j in range(4):
                        i = i0 + j
                        nc.vector.tensor_scalar(
                            out=sc4[:, j, :], in0=ia_ts[j][:],
                            scalar1=scale2d[:, i:i + 1], scalar2=None,
                            op0=ALU.mult,
                        )
                    nc.sync.dma_start(
                        out=tsc[i0 * 128:(i0 + 4) * 128, :], in_=sc4[:]
                    )
                    yield
                idt = idt_pool.tile([128, NDB, nt * 128], BF16, tag="idt")
                r0 = GSTART[g] * 128
                for db in range(NDB):
                    nc.sync.dma_start(
                        out=idt[:, db, :],
                        in_=tsc[r0:r0 + nt * 128, db * 128:(db + 1) * 128],
                        transpose=True,
                    )
                idts[g] = idt
                yield

            def mm_sweep(g, idt, interleave):
                """Matmul + exp sweep for group g, pulling stage-1 bundles of
                the next group between iterations. PSUM pairing: 512-class
                chunks share 2-bank psum tiles so one exp+accum covers 1024
                classes where possible."""
                for bt in range(BT):
                    pieces = []                          # (psum, width, ch0)
                    for c0, w in pieces_for(GROUPS[g]):
                        ps = ps_pool.tile([128, 1024], F32, tag="ps")
                        pieces.append((ps, w, c0))
                    for db in range(NDB):
                        for ps, w, c0 in pieces:
                            for k in range(w):
                                cof = (c0 + k) * CHUNK_T * 128
                                nc.tensor.matmul(
                                    out=ps[:, k * 512:(k + 1) * 512],
                                    lhsT=xT[:, db, bt * 128:(bt + 1) * 128],
                                    rhs=idt[:, db, cof:cof + CHUNK_T * 128],
                                    start=(db == 0), stop=(db == NDB - 1),
                                )
                    for pi, (ps, w, c0) in enumerate(pieces):
                        edump = dump_pool.tile([128, 1024], BF16, tag="edump")
                        col = bt * NPIECE + PBASE[g] + pi
                        nc.scalar.activation(
                            edump[:, :w * 512], ps[:, :w * 512], AF.Exp,
                            accum_out=zp2d[:, col:col + 1],
                        )
                    for _ in range(3):
                        next(interleave, None)

            def disc_gen():
                # ---------------- disc-loss gather path ----------------
                for bt in range(BT):
                    nc.gpsimd.indirect_dma_start(
                        out=g3[:, bt, :], out_offset=None,
                        in_=ia_d[:, :],
                        in_offset=bass.IndirectOffsetOnAxis(
                            ap=toffs[:, bt:bt + 1], axis=0
                        ),
                    )
                    nc.gpsimd.indirect_dma_start(
                        out=btg3[:, bt, :], out_offset=None,
                        in_=bsh_d[:, :],
                        in_offset=bass.IndirectOffsetOnAxis(
                            ap=toffs[:, bt:bt + 1], axis=0
                        ),
                    )
                    dmp = dump_pool.tile([128, D], F32, tag="dmpf32")
                    nc.vector.scalar_tensor_tensor(
                        out=dmp[:], in0=g3[:, bt, :], scalar=1.0,
                        in1=g3[:, bt, :], op0=ALU.mult, op1=ALU.mult,
                        accum_out=ng2[:, bt:bt + 1],
                    )
                    dmp = dump_pool.tile([128, D], F32, tag="dmpf32")
                    nc.vector.scalar_tensor_tensor(
                        out=dmp[:], in0=g3[:, bt, :], scalar=1.0,
                        in1=xn3[:, bt, :], op0=ALU.mult, op1=ALU.mult,
                        accum_out=dot8[:, bt:bt + 1],
                    )
                    dmp = dump_pool.tile([128, D], F32, tag="dmpf32")
                    nc.vector.scalar_tensor_tensor(
                        out=dmp[:], in0=btg3[:, bt, :], scalar=1.0,
                        in1=btg3[:, bt, :], op0=ALU.mult, op1=ALU.mult,
                        accum_out=btn2[:, bt:bt + 1],
                    )
                    yield
                # s1 = 1/||ia_t|| ; f = min(1, 0.05/||bt||)
                nc.vector.tensor_scalar_max(out=ng2[:], in0=ng2[:], scalar1=1e-30)
                nc.vector.tensor_scalar_max(out=btn2[:], in0=btn2[:], scalar1=1e-30)
                nc.scalar.activation(lb8[:], ng2[:], AF.Ln)
                nc.scalar.activation(s1_8[:], lb8[:], AF.Exp, scale=-0.5)
                nc.scalar.activation(lb8[:], btn2[:], AF.Ln)
                nc.scalar.activation(f8[:], lb8[:], AF.Exp, scale=-0.5, bias=LOG_BCLIP)
                nc.vector.tensor_scalar_min(out=f8[:], in0=f8[:], scalar1=1.0)
                yield
                for bt in range(BT):
                    t1 = w_pool.tile([128, D], F32, tag="wk")
                    nc.vector.scalar_tensor_tensor(
                        out=t1[:], in0=g3[:, bt, :], scalar=s1_8[:, bt:bt + 1],
                        in1=xn3[:, bt, :], op0=ALU.mult, op1=ALU.subtract,
                    )
                    t2 = w_pool.tile([128, D], F32, tag="wk")
                    dmp = dump_pool.tile([128, D], F32, tag="dmpf32")
                    nc.vector.scalar_tensor_tensor(
                        out=t2[:], in0=btg3[:, bt, :], scalar=f8[:, bt:bt + 1],
                        in1=t1[:], op0=ALU.mult, op1=ALU.add,
                    )
                    nc.vector.scalar_tensor_tensor(
                        out=dmp[:], in0=t2[:], scalar=1.0,
                        in1=t2[:], op0=ALU.mult, op1=ALU.mult,
                        accum_out=rn2[:, bt:bt + 1],
                    )
                    yield
                # rn = sqrt(rn2); st = dot * s1; payload cols 8:16 st, 16:24 rn
                nc.vector.tensor_scalar_max(out=rn2[:], in0=rn2[:], scalar1=1e-30)
                nc.scalar.activation(lb8[:], rn2[:], AF.Ln)
                nc.scalar.activation(lb8[:], lb8[:], AF.Exp, scale=0.5)
                nc.vector.tensor_tensor(
                    out=payload1[:, 8:16], in0=lb8[:], in1=tmasks[:], op=ALU.mult
                )
                nc.vector.tensor_tensor(
                    out=s1_8[:], in0=dot8[:], in1=s1_8[:], op=ALU.mult
                )
                nc.vector.tensor_tensor(
                    out=payload1[:, 0:8], in0=s1_8[:], in1=tmasks[:], op=ALU.mult
                )
                # early all-reduce of the disc-path payload; overlaps the main
                # loop, and lets e1/e2 be precomputed before Z lands.
                nc.gpsimd.dma_start(out=ccin1[:], in_=payload1[:])
                nc.gpsimd.collective_compute(
                    "AllReduce", ALU.add,
                    replica_groups=[list(range(NCORES))],
                    ins=[ccin1[:]], outs=[ccout1[:]],
                )
                nc.gpsimd.dma_start(out=allred1[:], in_=ccout1[:])
                e1 = s_pool.tile([128, 8], F32, tag="e1")
                e2 = s_pool.tile([128, 8], F32, tag="e2")
                eref["e1"], eref["e2"] = e1, e2
                nc.scalar.activation(e1[:], allred1[:, 0:8], AF.Exp, scale=SCALE)
                nc.scalar.activation(
                    e2[:], allred1[:, 0:8], AF.Exp, scale=SCALE, bias=-SM
                )
                yield


            eref = {}

            def producer_chain():
                # trickle disc-path bundles between stage-1 bundles so they
                # never gate group production or hog the gpsimd queue
                dg = disc_gen()
                for g in range(len(GROUPS)):
                    for _ in produce(g):
                        yield
                        if g >= 1:
                            next(dg, None)
                            yield
                for _ in dg:
                    yield

            prod = producer_chain()
            for g in range(len(GROUPS)):
                while g not in idts:
                    next(prod)
                mm_sweep(g, idts[g], prod)
            for _ in prod:
                pass

            # ---------------- reduce Z partials ----------------
            for bt in range(BT):
                nc.vector.reduce_sum(
                    out=payload2[:, bt:bt + 1],
                    in_=zp2d[:, bt * NPIECE:(bt + 1) * NPIECE],
                    axis=AX.X,
                )

            # ---------------- all-reduce of Z partials ----------------
            nc.gpsimd.dma_start(out=ccin2[:], in_=payload2[:])
            nc.gpsimd.collective_compute(
                "AllReduce", ALU.add,
                replica_groups=[list(range(NCORES))],
                ins=[ccin2[:]], outs=[ccout2[:]],
            )
            nc.gpsimd.dma_start(out=allred2[:], in_=ccout2[:])

            # ---------------- final loss math (identical on all cores) -------
            zsum = allred2[:, 0:8]
            st8 = allred1[:, 0:8]
            rn8 = allred1[:, 8:16]
            zc = s_pool.tile([128, 8], F32, tag="zc")
            lnz = s_pool.tile([128, 8], F32, tag="lnz")
            nll = s_pool.tile([128, 8], F32, tag="nll")
            nc.vector.tensor_scalar_add(
                out=zc[:], in0=zsum, scalar1=-NPAD_TOTAL
            )
            e1, e2 = eref["e1"], eref["e2"]
            nc.vector.tensor_tensor(out=zc[:], in0=zc[:], in1=e1[:], op=ALU.subtract)
            nc.vector.tensor_tensor(out=zc[:], in0=zc[:], in1=e2[:], op=ALU.add)
            nc.scalar.activation(lnz[:], zc[:], AF.Ln)
            # nll = lnz - 64*st + 22.4
            nc.vector.scalar_tensor_tensor(
                out=nll[:], in0=st8, scalar=-SCALE, in1=lnz[:],
                op0=ALU.mult, op1=ALU.add,
            )
            nc.vector.tensor_scalar_add(out=nll[:], in0=nll[:], scalar1=SM)
            red2 = s_pool.tile([128, 2], F32, tag="red2")
            nc.vector.reduce_sum(out=red2[:, 0:1], in_=nll[:], axis=AX.X)
            nc.vector.reduce_sum(out=red2[:, 1:2], in_=rn8, axis=AX.X)
            fin_ps = ps_pool.tile([128, 1024], F32, tag="ps")
            nc.tensor.matmul(
                out=fin_ps[0:1, 0:2], lhsT=ones[:], rhs=red2[:],
                start=True, stop=True,
            )
            fin = s_pool.tile([1, 2], F32, tag="fin")
            nc.vector.tensor_copy(out=fin[:], in_=fin_ps[0:1, 0:2])
            p_t = s_pool.tile([1, 1], F32, tag="p_t")
            nc.scalar.activation(p_t[:], fin[:, 0:1], AF.Exp, scale=-1.0 / B)
            q_t = s_pool.tile([1, 1], F32, tag="q_t")
            nc.vector.tensor_scalar(
                out=q_t[:], in0=p_t[:], scalar1=-1.0, scalar2=1.0,
                op0=ALU.mult, op1=ALU.add,
            )
            nc.vector.tensor_tensor(out=q_t[:], in0=q_t[:], in1=q_t[:], op=ALU.mult)
            lgp = s_pool.tile([1, 1], F32, tag="lgp")
            nc.vector.tensor_scalar_mul(out=lgp[:], in0=fin[:, 0:1], scalar1=1.0 / B)
            nc.vector.tensor_tensor(out=q_t[:], in0=q_t[:], in1=lgp[:], op=ALU.mult)
            rterm = s_pool.tile([1, 1], F32, tag="rterm")
            nc.vector.tensor_scalar_mul(
                out=rterm[:], in0=fin[:, 1:2], scalar1=LAMBDA / B
            )
            nc.vector.tensor_tensor(
                out=q_t[:], in0=q_t[:], in1=rterm[:], op=ALU.add
            )
            nc.gpsimd.dma_start(out=out_d[:], in_=q_t[:])

    n = split_multi_waits(nc)
    print(f"split_multi_waits: inserted {n} wait-nops")
    return nc


_NC_CACHE = {}


def _get_nc():
    if "nc" not in _NC_CACHE:
        _NC_CACHE["nc"] = build_bass()
    return _NC_CACHE["nc"]


def make_in_maps(x, target, id_agent, b):
    x = np.ascontiguousarray(np.asarray(x, dtype=np.float32))
    target = np.asarray(target).astype(np.int64)
    id_agent = np.asarray(id_agent, dtype=np.float32)
    b = np.asarray(b, dtype=np.float32)

    in_maps = []
    for k in range(NCORES):
        lo = k * CPER
        ia_k = np.zeros((CSH, D), dtype=np.float32)
        ia_k[:CPER] = id_agent[lo:lo + CPER]
        b_k = np.zeros((CSH, D), dtype=np.float32)
        b_k[:CPER] = b[lo:lo + CPER]
        tloc = np.clip(target - lo, 0, CPER - 1).astype(np.int32)
        owned = ((target >= lo) & (target < lo + CPER)).astype(np.float32)
        toff_k = np.ascontiguousarray(tloc.reshape(BT, 128).T)
        tmask_k = np.ascontiguousarray(owned.reshape(BT, 128).T)
        in_maps.append(
            {
                "x": x,
                "ia": ia_k,
                "bsh": b_k,
                "toff": toff_k,
                "tmask": tmask_k,
            }
        )
    return in_maps


def run(inputs, trace=False, **kw):
    nc = _get_nc()
    in_maps = make_in_maps(**inputs)
    res = run_bass_kernel_spmd(
        nc, in_maps, core_ids=list(range(NCORES)), trace=trace, **kw
    )
    return res


def kernel(x, target, id_agent, b):
    res = run({"x": x, "target": target, "id_agent": id_agent, "b": b})
    return np.asarray(res.results[0]["out"], dtype=np.float32)



# revision 5
# speedup vs baseline: 1.0475x; 1.0475x over previous
"""DiscFace AM-softmax loss kernel for 8 TRN2 NeuronCores.

Strategy (tensor-parallel over classes), v2 — fp8 DoubleRow edition:
  - id_agent/b sharded row-wise: core k owns classes [k*12500, (k+1)*12500),
    padded to 12800 rows with zeros (pad rows produce logits == 0 exactly,
    contributing exp(0) == 1 each to the softmax denominator; the constant
    8*300 = 2400 is subtracted during the final correction).
  - Weights path: ia rows stream in as [128p, 4r, 512d] fp32 (8KB-contiguous
    descriptors), DVE computes row sumsq, ACT computes 16/||row|| via
    Ln/Exp(bias=ln16), DVE scales+casts to fp8e4. The fp8 tile is viewed as
    u16 pairs and transposed SBUF->SBUF through the DMA xbar into
    pairT[p, j, c] (u16 = fp8 pair d=(256j+2p, 256j+2p+1)). No DRAM staging.
  - x replicated: xn3 = x/||x|| fp32 (disc path needs it), and
    xTw[p, j, ko, b] = fp8(64 * xn[b, 256j+2p+ko]) via PE transposes of
    strided xn columns.
  - GEMM: DoubleRow fp8 matmuls, contraction (Ki=p, Ko=pair byte), psum =
    (64 xn)·(16 w_unit) = 1024 cos; ACT exp(psum/16) with accum_out gives
    per-batch partial Z over 2048-class groups (4-bank-wide activations).
  - The margin on the target logit is applied via a scalar correction:
    Z += exp(64*st - 22.4) - exp(64*st), with st = cos(x_n, w_target)
    computed exactly (fp32) from an on-device indirect-DMA gather of the
    owned target rows (ownership-masked, clamped local indices).
  - One AllReduce of a [128, 24] payload (Z partials / masked st / masked
    residual norms), then every core finishes the focal + disc loss math;
    core 0's [1] output is returned.
  - Engine discipline: ACT does only Ln/Exp/Copy (single activation table,
    no ACT_TABLE_LOADs) and dispatches no DMAs; DMA dispatch lives on
    SP (xbar transposes, x loads) and gpsimd/Pool (ia streams, gathers,
    collectives).
"""

import os
import sys

import numpy as np

sys.path.insert(0, "/opt/trn_rl_repo")

from concourse import bass, mybir, tile  # noqa: E402
from concourse.bass_utils import run_bass_kernel_spmd  # noqa: E402

B, D, C = 1024, 512, 100000
NCORES = 8
CPER = C // NCORES          # 12500 real classes per core
CSH = 12800                 # padded shard rows (25 blocks of 512)
NPAD_TOTAL = float(NCORES * (CSH - CPER))   # 2400 pad contributions to Z
NB = CSH // 512             # 25 load-blocks (512 classes each)
BT = B // 128               # 8 batch tiles
# Consumer groups (in load-blocks): first small so the PE pipeline starts
# early; the rest 4-block (2048-class / 4-psum-bank) groups.
GROUPS = [1, 4, 4, 4, 4, 4, 4]
assert sum(GROUPS) == NB
GSTART = [sum(GROUPS[:g]) for g in range(len(GROUPS))]
NG = len(GROUPS)

SCALE = 64.0
MARGIN = 0.35
LAMBDA = 0.4
SM = SCALE * MARGIN         # 22.4
WSC = 16.0                  # fp8 weight scale: w = WSC * ia/||ia||
XSC = 64.0                  # fp8 x scale:     xq = XSC * xn
EXPSC = SCALE / (WSC * XSC)  # exp scale on psum -> 1/16
LOG_WSC = float(np.log(WSC))
LOG_BCLIP = float(np.log(0.05))

F32 = mybir.dt.float32
BF16 = mybir.dt.bfloat16
F8 = mybir.dt.float8e4
U16 = mybir.dt.uint16
I32 = mybir.dt.int32
AF = mybir.ActivationFunctionType
ALU = mybir.AluOpType
AX = mybir.AxisListType
DR = mybir.MatmulPerfMode.DoubleRow


# Engine-executed compute instruction classes. The TRN2 TPB instruction
# encoding has exactly ONE semaphore-wait slot, and walrus refuses to encode
# instructions carrying more. Tile's scheduler attaches as many waits as the
# dependency structure demands, so after scheduling we move every wait off
# compute instructions onto same-engine NoOps (one wait each).
_NO_SPLIT_CLASSES = ("InstISA", "InstCall")


def split_multi_waits(nc):
    n_nops = 0
    for f in nc.m.functions:
        for bb in f.blocks:
            new_insts = []
            for inst in bb.instructions:
                si = inst.sync_info
                cls = type(inst).__name__
                zero_wait = (
                    cls != "InstISA"
                    and (hasattr(inst, "isa_opcode") or cls == "InstDmaTransposeAnt")
                )
                keep = 0 if zero_wait else 1
                if (
                    si is not None
                    and len(si.on_wait) > keep
                    and cls not in _NO_SPLIT_CLASSES
                ):
                    split = si.on_wait[:-keep] if keep else list(si.on_wait)
                    for w in split:
                        nop = mybir.InstNoOp(
                            name=nc.get_next_instruction_name(),
                            sync_info=mybir.SyncInfo(on_wait=[w], on_update=[]),
                            bass_nofuse=True,
                            engine=inst.engine,
                        )
                        nc.inst_map[nop.name] = nop
                        new_insts.append(nop)
                        n_nops += 1
                    inst.sync_info = mybir.SyncInfo(
                        on_wait=list(si.on_wait[-keep:]) if keep else [],
                        on_update=list(si.on_update),
                    )
                new_insts.append(inst)
            bb.instructions = new_insts
    return n_nops


def build_bass():
    nc = bass.Bass(trn_type="TRN2", num_devices=NCORES)

    x_d = nc.declare_dram_parameter("x", [B, D], F32, isOutput=False)
    ia_d = nc.declare_dram_parameter("ia", [CSH, D], F32, isOutput=False)
    bsh_d = nc.declare_dram_parameter("bsh", [CSH, D], F32, isOutput=False)
    toff_d = nc.declare_dram_parameter("toff", [128, BT], I32, isOutput=False)
    tmask_d = nc.declare_dram_parameter("tmask", [128, BT], F32, isOutput=False)
    out_d = nc.declare_dram_parameter("out", [1], F32, isOutput=True)

    ccin1 = nc.dram_tensor("ccin1", [128, 16], F32)
    ccout1 = nc.dram_tensor("ccout1", [128, 16], F32, addr_space="Shared")
    ccin2 = nc.dram_tensor("ccin2", [128, BT], F32)
    ccout2 = nc.dram_tensor("ccout2", [128, BT], F32, addr_space="Shared")

    # Register const APs for the nonzero activation biases we use.
    for v in (LOG_WSC, LOG_BCLIP, -SM):
        t = nc.alloc_sbuf_tensor(f"const-f32-{v}", [128, 1], F32)
        nc.gpsimd.memset(t.ap(), v)
        nc.const_aps.aps[(F32, v)] = t.ap()
    nc.all_engine_barrier()

    with tile.TileContext(nc) as tc:
        with (
            tc.tile_pool(name="persist", bufs=1) as pp,
            tc.tile_pool(name="ia", bufs=4) as ia_pool,
            tc.tile_pool(name="scaled", bufs=3) as sc_pool,
            tc.tile_pool(name="dump", bufs=3) as dump_pool,
            tc.tile_pool(name="edump", bufs=2) as ed_pool,
            tc.tile_pool(name="work", bufs=3) as w_pool,
            tc.tile_pool(name="small", bufs=2) as s_pool,
            tc.tile_pool(name="psum", bufs=2, space="PSUM") as ps_pool,
        ):
            # ---------------- persistent tiles ----------------
            xn3 = pp.tile([128, BT, D], F32, tag="xn3")          # normalized x
            xTw = pp.tile([128, 2, 2, B], F8, tag="xTw")         # [p, j, ko, b]
            ssx = pp.tile([128, BT], F32, tag="ssx")
            xscale = pp.tile([128, BT], F32, tag="xscale")
            ss2d = pp.tile([128, 4 * NB], F32, tag="ss2d")       # row sumsq
            scale2d = pp.tile([128, 4 * NB], F32, tag="scale2d")  # 16/norm
            pairT = pp.tile([128, 2, CSH], U16, tag="pairT")     # fp8 pairs
            zp2d = pp.tile([128, NG * BT], F32, tag="zp2d")      # exp partials
            payload1 = pp.tile([128, 16], F32, tag="payload1")
            payload2 = pp.tile([128, BT], F32, tag="payload2")
            allred1 = pp.tile([128, 16], F32, tag="allred1")
            allred2 = pp.tile([128, BT], F32, tag="allred2")
            toffs = pp.tile([128, BT], I32, tag="toffs")
            tmasks = pp.tile([128, BT], F32, tag="tmasks")
            ones = pp.tile([128, 1], F32, tag="ones")
            ident = pp.tile([128, 128], F32, tag="ident")
            # disc-path persistents
            ng2 = pp.tile([128, BT], F32, tag="ng2")
            dot8 = pp.tile([128, BT], F32, tag="dot8")
            btn2 = pp.tile([128, BT], F32, tag="btn2")
            rn2 = pp.tile([128, BT], F32, tag="rn2")
            s1_8 = pp.tile([128, BT], F32, tag="s1_8")
            f8t = pp.tile([128, BT], F32, tag="f8t")
            lb8 = pp.tile([128, BT], F32, tag="lb8")
            g3 = pp.tile([128, BT, D], F32, tag="g3")            # gathered ia rows
            btg3 = pp.tile([128, BT, D], F32, tag="btg3")        # gathered b rows

            nc.vector.memset(ones[:], 1.0)
            from concourse.masks import make_identity
            make_identity(nc, ident[:])

            # fp8 view of pairT: [p, j, ko, class]
            pairT_f8 = pairT[:].bitcast(F8).rearrange(
                "p j (c k) -> p j k c", k=2
            )

            # ---------------- phase 0: x normalize + transpose ----------------
            nc.gpsimd.dma_start(out=toffs[:], in_=toff_d[:])
            nc.gpsimd.dma_start(out=tmasks[:], in_=tmask_d[:])

            for bt in range(BT):
                nc.sync.dma_start(
                    out=xn3[:, bt, :], in_=x_d[bt * 128:(bt + 1) * 128, :]
                )
                dmp = dump_pool.tile([128, D], F32, tag="dmpf32")
                nc.vector.scalar_tensor_tensor(
                    out=dmp[:], in0=xn3[:, bt, :], scalar=1.0,
                    in1=xn3[:, bt, :], op0=ALU.mult, op1=ALU.mult,
                    accum_out=ssx[:, bt:bt + 1],
                )
            # xscale = exp(-0.5 * log(ssx)) = 1/||x||
            nc.vector.tensor_scalar_max(out=ssx[:], in0=ssx[:], scalar1=1e-30)
            nc.scalar.activation(xscale[:], ssx[:], AF.Ln)
            nc.scalar.activation(xscale[:], xscale[:], AF.Exp, scale=-0.5)
            for bt in range(BT):
                nc.vector.tensor_scalar_mul(
                    out=xn3[:, bt, :], in0=xn3[:, bt, :],
                    scalar1=xscale[:, bt:bt + 1],
                )
                # PE transposes of strided xn columns -> xTw fp8 (x64)
                tp = ps_pool.tile([128, 2048], F32, tag="ps")
                for j in range(2):
                    for ko in range(2):
                        nc.tensor.transpose(
                            out=tp[:, (2 * j + ko) * 128:(2 * j + ko + 1) * 128],
                            in_=xn3[:, bt, 256 * j + ko:256 * (j + 1):2],
                            identity=ident[:],
                        )
                nc.vector.tensor_scalar(
                    out=xTw[:, :, :, bt * 128:(bt + 1) * 128],
                    in0=tp[:, 0:512].rearrange("p (j k b) -> p j k b", j=2, k=2),
                    scalar1=XSC, scalar2=None, op0=ALU.mult,
                )

            # ---------------- main class loop ----------------
            # Producer: per load-block k, stream [128, 4, 512] fp32 rows,
            # sumsq -> 16/||row|| -> fp8 cast -> 4x u16 xbar transpose into
            # pairT columns [k*512, (k+1)*512).
            # Consumer: per group g (GROUPS[g] blocks), per batch tile:
            # DoubleRow fp8 matmuls j=0/1 into a 4-bank psum tile, then one
            # wide exp+accum into zp2d.

            produced = [False] * NB

            def produce(k):
                """Emit the full produce pipeline for load-block k as an
                interleavable generator."""
                ia4 = ia_pool.tile([128, 4, D], F32, tag="ia4")
                nc.gpsimd.dma_start(
                    out=ia4[:], in_=ia_d[k * 512:(k + 1) * 512, :]
                )
                for r in range(4):
                    dmp = dump_pool.tile([128, D], F32, tag="dmpf32")
                    nc.vector.scalar_tensor_tensor(
                        out=dmp[:], in0=ia4[:, r, :], scalar=1.0,
                        in1=ia4[:, r, :], op0=ALU.mult, op1=ALU.mult,
                        accum_out=ss2d[:, 4 * k + r:4 * k + r + 1],
                    )
                yield
                lbuf = s_pool.tile([128, 4], F32, tag="lbuf")
                nc.vector.tensor_scalar_max(
                    out=lbuf[:], in0=ss2d[:, 4 * k:4 * k + 4], scalar1=1e-30,
                )
                nc.scalar.activation(lbuf[:], lbuf[:], AF.Ln)
                nc.scalar.activation(
                    scale2d[:, 4 * k:4 * k + 4], lbuf[:], AF.Exp,
                    scale=-0.5, bias=LOG_WSC,
                )
                scq = sc_pool.tile([128, 4, D], F8, tag="scq")
                for r in range(4):
                    nc.vector.tensor_scalar(
                        out=scq[:, r, :], in0=ia4[:, r, :],
                        scalar1=scale2d[:, 4 * k + r:4 * k + r + 1],
                        scalar2=None, op0=ALU.mult,
                    )
                yield
                scq_u16 = scq[:].bitcast(U16)  # [128, 4, 256]
                for r in range(4):
                    nc.sync.dma_start_transpose(
                        out=pairT[:, :, (4 * k + r) * 128:(4 * k + r + 1) * 128],
                        in_=scq_u16[:, r, :],
                    )
                produced[k] = True
                yield

            def mm_sweep(g, interleave):
                """Matmul + exp sweep for group g, pulling producer bundles
                between batch tiles."""
                gw = GROUPS[g]
                k0 = GSTART[g]
                for bt in range(BT):
                    ps = ps_pool.tile([128, 2048], F32, tag="ps")
                    for j in range(2):
                        for kk in range(gw):
                            nc.tensor.matmul(
                                out=ps[:, kk * 512:(kk + 1) * 512],
                                lhsT=xTw[:, j, :, bt * 128:(bt + 1) * 128],
                                rhs=pairT_f8[
                                    :, j, :, (k0 + kk) * 512:(k0 + kk + 1) * 512
                                ],
                                start=(j == 0), stop=(j == 1),
                                perf_mode=DR,
                            )
                    edump = ed_pool.tile([128, 2048], BF16, tag="edump")
                    nc.scalar.activation(
                        edump[:, :gw * 512], ps[:, :gw * 512], AF.Exp,
                        scale=EXPSC,
                        accum_out=zp2d[:, g * BT + bt:g * BT + bt + 1],
                    )
                    for _ in range(3):
                        next(interleave, None)

            def disc_gen():
                # ---------------- disc-loss gather path ----------------
                for bt in range(BT):
                    nc.gpsimd.indirect_dma_start(
                        out=g3[:, bt, :], out_offset=None,
                        in_=ia_d[:, :],
                        in_offset=bass.IndirectOffsetOnAxis(
                            ap=toffs[:, bt:bt + 1], axis=0
                        ),
                    )
                    nc.gpsimd.indirect_dma_start(
                        out=btg3[:, bt, :], out_offset=None,
                        in_=bsh_d[:, :],
                        in_offset=bass.IndirectOffsetOnAxis(
                            ap=toffs[:, bt:bt + 1], axis=0
                        ),
                    )
                    dmp = dump_pool.tile([128, D], F32, tag="dmpf32")
                    nc.vector.scalar_tensor_tensor(
                        out=dmp[:], in0=g3[:, bt, :], scalar=1.0,
                        in1=g3[:, bt, :], op0=ALU.mult, op1=ALU.mult,
                        accum_out=ng2[:, bt:bt + 1],
                    )
                    dmp = dump_pool.tile([128, D], F32, tag="dmpf32")
                    nc.vector.scalar_tensor_tensor(
                        out=dmp[:], in0=g3[:, bt, :], scalar=1.0,
                        in1=xn3[:, bt, :], op0=ALU.mult, op1=ALU.mult,
                        accum_out=dot8[:, bt:bt + 1],
                    )
                    dmp = dump_pool.tile([128, D], F32, tag="dmpf32")
                    nc.vector.scalar_tensor_tensor(
                        out=dmp[:], in0=btg3[:, bt, :], scalar=1.0,
                        in1=btg3[:, bt, :], op0=ALU.mult, op1=ALU.mult,
                        accum_out=btn2[:, bt:bt + 1],
                    )
                    yield
                # s1 = 1/||ia_t|| ; f = min(1, 0.05/||bt||)
                nc.vector.tensor_scalar_max(out=ng2[:], in0=ng2[:], scalar1=1e-30)
                nc.vector.tensor_scalar_max(out=btn2[:], in0=btn2[:], scalar1=1e-30)
                nc.scalar.activation(lb8[:], ng2[:], AF.Ln)
                nc.scalar.activation(s1_8[:], lb8[:], AF.Exp, scale=-0.5)
                nc.scalar.activation(lb8[:], btn2[:], AF.Ln)
                nc.scalar.activation(f8t[:], lb8[:], AF.Exp, scale=-0.5, bias=LOG_BCLIP)
                nc.vector.tensor_scalar_min(out=f8t[:], in0=f8t[:], scalar1=1.0)
                yield
                for bt in range(BT):
                    t1 = w_pool.tile([128, D], F32, tag="wk")
                    nc.vector.scalar_tensor_tensor(
                        out=t1[:], in0=g3[:, bt, :], scalar=s1_8[:, bt:bt + 1],
                        in1=xn3[:, bt, :], op0=ALU.mult, op1=ALU.subtract,
                    )
                    t2 = w_pool.tile([128, D], F32, tag="wk")
                    dmp = dump_pool.tile([128, D], F32, tag="dmpf32")
                    nc.vector.scalar_tensor_tensor(
                        out=t2[:], in0=btg3[:, bt, :], scalar=f8t[:, bt:bt + 1],
                        in1=t1[:], op0=ALU.mult, op1=ALU.add,
                    )
                    nc.vector.scalar_tensor_tensor(
                        out=dmp[:], in0=t2[:], scalar=1.0,
                        in1=t2[:], op0=ALU.mult, op1=ALU.mult,
                        accum_out=rn2[:, bt:bt + 1],
                    )
                    yield
                # rn = sqrt(rn2); st = dot * s1; payload cols 8:16 st, 16:24 rn
                nc.vector.tensor_scalar_max(out=rn2[:], in0=rn2[:], scalar1=1e-30)
                nc.scalar.activation(lb8[:], rn2[:], AF.Ln)
                nc.scalar.activation(lb8[:], lb8[:], AF.Exp, scale=0.5)
                nc.vector.tensor_tensor(
                    out=payload1[:, 8:16], in0=lb8[:], in1=tmasks[:], op=ALU.mult
                )
                nc.vector.tensor_tensor(
                    out=s1_8[:], in0=dot8[:], in1=s1_8[:], op=ALU.mult
                )
                nc.vector.tensor_tensor(
                    out=payload1[:, 0:8], in0=s1_8[:], in1=tmasks[:], op=ALU.mult
                )
                # early all-reduce of the disc-path payload; overlaps the main
                # loop, and lets e1/e2 be precomputed before Z lands.
                nc.gpsimd.dma_start(out=ccin1[:], in_=payload1[:])
                nc.gpsimd.collective_compute(
                    "AllReduce", ALU.add,
                    replica_groups=[list(range(NCORES))],
                    ins=[ccin1[:]], outs=[ccout1[:]],
                )
                nc.gpsimd.dma_start(out=allred1[:], in_=ccout1[:])
                e1 = s_pool.tile([128, 8], F32, tag="e1")
                e2 = s_pool.tile([128, 8], F32, tag="e2")
                eref["e1"], eref["e2"] = e1, e2
                nc.scalar.activation(e1[:], allred1[:, 0:8], AF.Exp, scale=SCALE)
                nc.scalar.activation(
                    e2[:], allred1[:, 0:8], AF.Exp, scale=SCALE, bias=-SM
                )
                yield

            eref = {}

            def producer_chain():
                # trickle disc-path bundles between produce bundles so they
                # never gate block production or hog the gpsimd queue
                dg = disc_gen()
                for k in range(NB):
                    for _ in produce(k):
                        yield
                        if k >= 1:
                            next(dg, None)
                            yield
                for _ in dg:
                    yield

            prod = producer_chain()
            for g in range(NG):
                while not all(produced[GSTART[g]:GSTART[g] + GROUPS[g]]):
                    next(prod)
                mm_sweep(g, prod)
            for _ in prod:
                pass

            # ---------------- reduce Z partials ----------------
            for bt in range(BT):
                nc.vector.reduce_sum(
                    out=payload2[:, bt:bt + 1],
                    in_=zp2d[:, bt::BT],
                    axis=AX.X,
                )

            # ---------------- all-reduce of Z partials ----------------
            nc.gpsimd.dma_start(out=ccin2[:], in_=payload2[:])
            nc.gpsimd.collective_compute(
                "AllReduce", ALU.add,
                replica_groups=[list(range(NCORES))],
                ins=[ccin2[:]], outs=[ccout2[:]],
            )
            nc.gpsimd.dma_start(out=allred2[:], in_=ccout2[:])

            # ---------------- final loss math (identical on all cores) -------
            zsum = allred2[:, 0:8]
            st8 = allred1[:, 0:8]
            rn8 = allred1[:, 8:16]
            zc = s_pool.tile([128, 8], F32, tag="zc")
            lnz = s_pool.tile([128, 8], F32, tag="lnz")
            nll = s_pool.tile([128, 8], F32, tag="nll")
            nc.vector.tensor_scalar_add(
                out=zc[:], in0=zsum, scalar1=-NPAD_TOTAL
            )
            e1, e2 = eref["e1"], eref["e2"]
            nc.vector.tensor_tensor(out=zc[:], in0=zc[:], in1=e1[:], op=ALU.subtract)
            nc.vector.tensor_tensor(out=zc[:], in0=zc[:], in1=e2[:], op=ALU.add)
            nc.scalar.activation(lnz[:], zc[:], AF.Ln)
            # nll = lnz - 64*st + 22.4
            nc.vector.scalar_tensor_tensor(
                out=nll[:], in0=st8, scalar=-SCALE, in1=lnz[:],
                op0=ALU.mult, op1=ALU.add,
            )
            nc.vector.tensor_scalar_add(out=nll[:], in0=nll[:], scalar1=SM)
            red2 = s_pool.tile([128, 2], F32, tag="red2")
            nc.vector.reduce_sum(out=red2[:, 0:1], in_=nll[:], axis=AX.X)
            nc.vector.reduce_sum(out=red2[:, 1:2], in_=rn8, axis=AX.X)
            fin_ps = ps_pool.tile([128, 2048], F32, tag="ps")
            nc.tensor.matmul(
                out=fin_ps[0:1, 0:2], lhsT=ones[:], rhs=red2[:],
                start=True, stop=True,
            )
            fin = s_pool.tile([1, 2], F32, tag="fin")
            nc.vector.tensor_copy(out=fin[:], in_=fin_ps[0:1, 0:2])
            p_t = s_pool.tile([1, 1], F32, tag="p_t")
            nc.scalar.activation(p_t[:], fin[:, 0:1], AF.Exp, scale=-1.0 / B)
            q_t = s_pool.tile([1, 1], F32, tag="q_t")
            nc.vector.tensor_scalar(
                out=q_t[:], in0=p_t[:], scalar1=-1.0, scalar2=1.0,
                op0=ALU.mult, op1=ALU.add,
            )
            nc.vector.tensor_tensor(out=q_t[:], in0=q_t[:], in1=q_t[:], op=ALU.mult)
            lgp = s_pool.tile([1, 1], F32, tag="lgp")
            nc.vector.tensor_scalar_mul(out=lgp[:], in0=fin[:, 0:1], scalar1=1.0 / B)
            nc.vector.tensor_tensor(out=q_t[:], in0=q_t[:], in1=lgp[:], op=ALU.mult)
            rterm = s_pool.tile([1, 1], F32, tag="rterm")
            nc.vector.tensor_scalar_mul(
                out=rterm[:], in0=fin[:, 1:2], scalar1=LAMBDA / B
            )
            nc.vector.tensor_tensor(
                out=q_t[:], in0=q_t[:], in1=rterm[:], op=ALU.add
            )
            nc.gpsimd.dma_start(out=out_d[:], in_=q_t[:])

    n = split_multi_waits(nc)
    print(f"split_multi_waits: inserted {n} wait-nops")
    return nc


_NC_CACHE = {}


def _get_nc():
    if "nc" not in _NC_CACHE:
        _NC_CACHE["nc"] = build_bass()
    return _NC_CACHE["nc"]


def make_in_maps(x, target, id_agent, b):
    x = np.ascontiguousarray(np.asarray(x, dtype=np.float32))
    target = np.asarray(target).astype(np.int64)
    id_agent = np.asarray(id_agent, dtype=np.float32)
    b = np.asarray(b, dtype=np.float32)

    in_maps = []
    for k in range(NCORES):
        lo = k * CPER
        ia_k = np.zeros((CSH, D), dtype=np.float32)
        ia_k[:CPER] = id_agent[lo:lo + CPER]
        b_k = np.zeros((CSH, D), dtype=np.float32)
        b_k[:CPER] = b[lo:lo + CPER]
        tloc = np.clip(target - lo, 0, CPER - 1).astype(np.int32)
        owned = ((target >= lo) & (target < lo + CPER)).astype(np.float32)
        toff_k = np.ascontiguousarray(tloc.reshape(BT, 128).T)
        tmask_k = np.ascontiguousarray(owned.reshape(BT, 128).T)
        in_maps.append(
            {
                "x": x,
                "ia": ia_k,
                "bsh": b_k,
                "toff": toff_k,
                "tmask": tmask_k,
            }
        )
    return in_maps


def run(inputs, trace=False, **kw):
    nc = _get_nc()
    in_maps = make_in_maps(**inputs)
    res = run_bass_kernel_spmd(
        nc, in_maps, core_ids=list(range(NCORES)), trace=trace, **kw
    )
    return res


def kernel(x, target, id_agent, b):
    res = run({"x": x, "target": target, "id_agent": id_agent, "b": b})
    return np.asarray(res.results[0]["out"], dtype=np.float32)


# revision 7
# speedup vs baseline: 1.4823x; 1.4151x over previous
"""DiscFace AM-softmax loss kernel for 8 TRN2 NeuronCores.

Strategy (tensor-parallel over classes), v3 — fp8 DoubleRow + bundled
single-transpose production:
  - id_agent/b sharded row-wise: core k owns classes [k*12500, (k+1)*12500),
    padded to 12800 rows with zeros (pad rows produce logits == 0 exactly,
    contributing exp(0) == 1 each to the softmax denominator; the constant
    8*300 = 2400 is subtracted during the final correction).
  - Production, per 1024-class bundle: one gpsimd cast-DMA streams rows as
    [128p, 8rr, 512d] bf16 (fp32 HBM read, bf16 SBUF write); DVE computes
    row sumsq per rr; ACT computes 16/||row|| via Ln/Exp(bias=ln16); DVE
    scales+casts to fp8 writing a (j, q, ko)-strided scq layout so that ONE
    u16 xbar transpose per bundle lands pairT[p, j, class] with classes
    contiguous. u16 u = fp8 pair d=(256j+2p, 256j+2p+1); class columns come
    out permuted (class = 8c+rr), which is irrelevant: Z sums over classes.
  - x replicated: xn3 = x/||x|| fp32 (disc path needs it), and
    xTw[p, j, ko, b] = fp8(64 * xn[b, 256j+2p+ko]) via PE transposes of
    strided xn columns.
  - GEMM: DoubleRow fp8 matmuls, psum = (64 xn)·(16 w_unit) = 1024 cos;
    ACT exp(psum/16) with accum_out gives per-batch partial Z over groups
    of up to 2048 classes (4-bank-wide activations).
  - The margin on the target logit is applied via a scalar correction:
    Z += exp(64*st - 22.4) - exp(64*st), with st = cos(x_n, w_target)
    computed exactly (fp32) from an on-device indirect-DMA gather of the
    owned target rows (ownership-masked, clamped local indices).
  - One AllReduce of a [128, 24] payload (Z partials / masked st / masked
    residual norms), then every core finishes the focal + disc loss math;
    core 0's [1] output is returned.
  - Engine discipline: ACT does only Ln/Exp/Copy (single activation table)
    and dispatches no DMAs; DMA dispatch lives on SP (xbar transposes,
    x loads) and gpsimd/Pool (ia streams, gathers, collectives).
"""

import os
import sys

import numpy as np

sys.path.insert(0, "/opt/trn_rl_repo")

from concourse import bass, mybir, tile  # noqa: E402
from concourse.bass_utils import run_bass_kernel_spmd  # noqa: E402

B, D, C = 1024, 512, 100000
NCORES = 8
CPER = C // NCORES          # 12500 real classes per core
CSH = 12800                 # padded shard rows
NPAD_TOTAL = float(NCORES * (CSH - CPER))   # 2400 pad contributions to Z
BT = B // 128               # 8 batch tiles

USE_BF16_LOADS = True       # gpsimd cast-DMA fp32->bf16 for the ia stream

# Bundles: 12 x 1024 classes + 1 x 512 classes (the tail holds the pads).
BUNDLES = [8] * 12 + [4]    # rr slices (128 classes each) per bundle
NBU = len(BUNDLES)
BSTART = [sum(BUNDLES[:i]) * 128 for i in range(NBU)]  # class offset
# Consumer groups as lists of (bundle, chunk-within-bundle) 512-class chunks.
GROUPS = [
    [(0, 0), (0, 1)],
    [(1, 0), (1, 1)],
    [(2, 0), (2, 1), (3, 0), (3, 1)],
    [(4, 0), (4, 1), (5, 0), (5, 1)],
    [(6, 0), (6, 1), (7, 0), (7, 1)],
    [(8, 0), (8, 1), (9, 0), (9, 1)],
    [(10, 0), (10, 1), (11, 0), (11, 1)],
    [(12, 0)],
]
NG = len(GROUPS)
assert sum(len(g) for g in GROUPS) * 512 == CSH

SCALE = 64.0
MARGIN = 0.35
LAMBDA = 0.4
SM = SCALE * MARGIN         # 22.4
WSC = 16.0                  # fp8 weight scale: w = WSC * ia/||ia||
XSC = 64.0                  # fp8 x scale:     xq = XSC * xn
EXPSC = SCALE / (WSC * XSC)  # exp scale on psum -> 1/16
LOG_WSC = float(np.log(WSC))
LOG_BCLIP = float(np.log(0.05))

F32 = mybir.dt.float32
BF16 = mybir.dt.bfloat16
F8 = mybir.dt.float8e4
U16 = mybir.dt.uint16
I32 = mybir.dt.int32
AF = mybir.ActivationFunctionType
ALU = mybir.AluOpType
AX = mybir.AxisListType
DR = mybir.MatmulPerfMode.DoubleRow
LOAD_DT = BF16 if USE_BF16_LOADS else F32


# The TRN2 TPB instruction encoding has exactly ONE semaphore-wait slot;
# move extra waits onto same-engine NoOps after scheduling.
_NO_SPLIT_CLASSES = ("InstISA", "InstCall")


def split_multi_waits(nc):
    n_nops = 0
    for f in nc.m.functions:
        for bb in f.blocks:
            new_insts = []
            for inst in bb.instructions:
                si = inst.sync_info
                cls = type(inst).__name__
                zero_wait = (
                    cls != "InstISA"
                    and (hasattr(inst, "isa_opcode") or cls == "InstDmaTransposeAnt")
                )
                keep = 0 if zero_wait else 1
                if (
                    si is not None
                    and len(si.on_wait) > keep
                    and cls not in _NO_SPLIT_CLASSES
                ):
                    split = si.on_wait[:-keep] if keep else list(si.on_wait)
                    for w in split:
                        nop = mybir.InstNoOp(
                            name=nc.get_next_instruction_name(),
                            sync_info=mybir.SyncInfo(on_wait=[w], on_update=[]),
                            bass_nofuse=True,
                            engine=inst.engine,
                        )
                        nc.inst_map[nop.name] = nop
                        new_insts.append(nop)
                        n_nops += 1
                    inst.sync_info = mybir.SyncInfo(
                        on_wait=list(si.on_wait[-keep:]) if keep else [],
                        on_update=list(si.on_update),
                    )
                new_insts.append(inst)
            bb.instructions = new_insts
    return n_nops


def build_bass():
    nc = bass.Bass(trn_type="TRN2", num_devices=NCORES)

    x_d = nc.declare_dram_parameter("x", [B, D], F32, isOutput=False)
    ia_d = nc.declare_dram_parameter("ia", [CSH, D], F32, isOutput=False)
    bsh_d = nc.declare_dram_parameter("bsh", [CSH, D], F32, isOutput=False)
    toff_d = nc.declare_dram_parameter("toff", [128, BT], I32, isOutput=False)
    tmask_d = nc.declare_dram_parameter("tmask", [128, BT], F32, isOutput=False)
    out_d = nc.declare_dram_parameter("out", [1], F32, isOutput=True)

    ccin1 = nc.dram_tensor("ccin1", [128, 16], F32)
    ccout1 = nc.dram_tensor("ccout1", [128, 16], F32, addr_space="Shared")
    ccin2 = nc.dram_tensor("ccin2", [128, BT], F32)
    ccout2 = nc.dram_tensor("ccout2", [128, BT], F32, addr_space="Shared")

    for v in (LOG_WSC, LOG_BCLIP, -SM):
        t = nc.alloc_sbuf_tensor(f"const-f32-{v}", [128, 1], F32)
        nc.gpsimd.memset(t.ap(), v)
        nc.const_aps.aps[(F32, v)] = t.ap()
    nc.all_engine_barrier()

    with tile.TileContext(nc) as tc:
        with (
            tc.tile_pool(name="persist", bufs=1) as pp,
            tc.tile_pool(name="ia", bufs=4) as ia_pool,
            tc.tile_pool(name="scaled", bufs=3) as sc_pool,
            tc.tile_pool(name="pairT", bufs=6) as pt_pool,
            tc.tile_pool(name="dump", bufs=3) as dump_pool,
            tc.tile_pool(name="edump", bufs=2) as ed_pool,
            tc.tile_pool(name="work", bufs=3) as w_pool,
            tc.tile_pool(name="small", bufs=2) as s_pool,
            tc.tile_pool(name="psum", bufs=2, space="PSUM") as ps_pool,
        ):
            # ---------------- persistent tiles ----------------
            xn3 = pp.tile([128, BT, D], F32, tag="xn3")          # normalized x
            xTw = pp.tile([128, 2, 2, B], F8, tag="xTw")         # [p, j, ko, b]
            ssx = pp.tile([128, BT], F32, tag="ssx")
            xscale = pp.tile([128, BT], F32, tag="xscale")
            ss2d = pp.tile([128, 8 * NBU], F32, tag="ss2d")      # row sumsq
            scale2d = pp.tile([128, 8 * NBU], F32, tag="scale2d")  # 16/norm
            zp2d = pp.tile([128, BT * NG], F32, tag="zp2d")      # exp partials
            payload1 = pp.tile([128, 16], F32, tag="payload1")
            payload2 = pp.tile([128, BT], F32, tag="payload2")
            allred1 = pp.tile([128, 16], F32, tag="allred1")
            allred2 = pp.tile([128, BT], F32, tag="allred2")
            toffs = pp.tile([128, BT], I32, tag="toffs")
            tmasks = pp.tile([128, BT], F32, tag="tmasks")
            ones = pp.tile([128, 1], F32, tag="ones")
            ident = pp.tile([128, 128], F32, tag="ident")
            # disc-path persistents
            ng2 = pp.tile([128, BT], F32, tag="ng2")
            dot8 = pp.tile([128, BT], F32, tag="dot8")
            btn2 = pp.tile([128, BT], F32, tag="btn2")
            rn2 = pp.tile([128, BT], F32, tag="rn2")
            s1_8 = pp.tile([128, BT], F32, tag="s1_8")
            f8t = pp.tile([128, BT], F32, tag="f8t")
            lb8 = pp.tile([128, BT], F32, tag="lb8")
            g3 = pp.tile([128, BT, D], F32, tag="g3")            # gathered ia rows
            btg3 = pp.tile([128, BT, D], F32, tag="btg3")        # gathered b rows

            nc.vector.memset(ones[:], 1.0)
            from concourse.masks import make_identity
            make_identity(nc, ident[:])

            # ---------------- phase 0: x normalize + transpose ----------------
            nc.gpsimd.dma_start(out=toffs[:], in_=toff_d[:])
            nc.gpsimd.dma_start(out=tmasks[:], in_=tmask_d[:])

            for bt in range(BT):
                nc.sync.dma_start(
                    out=xn3[:, bt, :], in_=x_d[bt * 128:(bt + 1) * 128, :]
                )
                dmp = dump_pool.tile([128, D], F32, tag="dmpf32")
                nc.vector.scalar_tensor_tensor(
                    out=dmp[:], in0=xn3[:, bt, :], scalar=1.0,
                    in1=xn3[:, bt, :], op0=ALU.mult, op1=ALU.mult,
                    accum_out=ssx[:, bt:bt + 1],
                )
            nc.vector.tensor_scalar_max(out=ssx[:], in0=ssx[:], scalar1=1e-30)
            nc.scalar.activation(xscale[:], ssx[:], AF.Ln)
            nc.scalar.activation(xscale[:], xscale[:], AF.Exp, scale=-0.5)
            for bt in range(BT):
                nc.vector.tensor_scalar_mul(
                    out=xn3[:, bt, :], in0=xn3[:, bt, :],
                    scalar1=xscale[:, bt:bt + 1],
                )
                tp = ps_pool.tile([128, 2048], F32, tag="ps")
                for j in range(2):
                    for ko in range(2):
                        nc.tensor.transpose(
                            out=tp[:, (2 * j + ko) * 128:(2 * j + ko + 1) * 128],
                            in_=xn3[:, bt, 256 * j + ko:256 * (j + 1):2],
                            identity=ident[:],
                        )
                nc.vector.tensor_scalar(
                    out=xTw[:, :, :, bt * 128:(bt + 1) * 128],
                    in0=tp[:, 0:512].rearrange("p (j k b) -> p j k b", j=2, k=2),
                    scalar1=XSC, scalar2=None, op0=ALU.mult,
                )

            # ---------------- main class loop ----------------
            pairTs = {}     # bundle -> fp8 AP view [p, j, ko, class]
            produced = [False] * NBU

            def produce(k):
                """Produce bundle k: stream, sumsq, scale, fp8 cast (strided
                layout), one u16 xbar transpose. Interleavable generator."""
                nrr = BUNDLES[k]
                c0 = BSTART[k]
                ia8 = ia_pool.tile([128, 8, D], LOAD_DT, tag="ia8")
                nc.gpsimd.dma_start(
                    out=ia8[:, :nrr, :], in_=ia_d[c0:c0 + nrr * 128, :]
                )
                for r0 in range(0, nrr, 4):
                    for rr in range(r0, r0 + 4):
                        dmp = dump_pool.tile([128, D], F32, tag="dmpf32")
                        nc.vector.scalar_tensor_tensor(
                            out=dmp[:], in0=ia8[:, rr, :], scalar=1.0,
                            in1=ia8[:, rr, :], op0=ALU.mult, op1=ALU.mult,
                            accum_out=ss2d[:, 8 * k + rr:8 * k + rr + 1],
                        )
                    yield
                lbuf = s_pool.tile([128, 8], F32, tag="lbuf")
                nc.vector.tensor_scalar_max(
                    out=lbuf[:, :nrr], in0=ss2d[:, 8 * k:8 * k + nrr],
                    scalar1=1e-30,
                )
                nc.scalar.activation(lbuf[:, :nrr], lbuf[:, :nrr], AF.Ln)
                nc.scalar.activation(
                    scale2d[:, 8 * k:8 * k + nrr], lbuf[:, :nrr], AF.Exp,
                    scale=-0.5, bias=LOG_WSC,
                )
                yield
                # scq layout per partition: [j][rr][q*2+ko] fp8
                scq = sc_pool.tile([128, 2, 8, 256], F8, tag="scq")
                for r0 in range(0, nrr, 4):
                    for rr in range(r0, r0 + 4):
                        nc.vector.tensor_scalar(
                            out=scq[:, :, rr, :].rearrange(
                                "p j (q t) -> p j q t", t=2
                            ),
                            in0=ia8[:, rr, :],
                            scalar1=scale2d[:, 8 * k + rr:8 * k + rr + 1],
                            scalar2=None, op0=ALU.mult,
                        )
                    yield
                pt = pt_pool.tile([128, 2, 1024], U16, tag="pairT")
                if nrr == 8:
                    nc.sync.dma_start_transpose(
                        out=pt[:].rearrange("p j (m c) -> p (j m) c", m=8),
                        in_=scq[:].bitcast(U16).rearrange("p j r u -> p (j r u)"),
                    )
                else:
                    # tail: only rr<4 slices are populated; the j-dim stride
                    # doesn't merge, so transpose each j half separately.
                    for j in range(2):
                        nc.sync.dma_start_transpose(
                            out=pt[:, j, :512].rearrange(
                                "p (m c) -> p m c", m=4
                            ),
                            in_=scq[:, j, :4, :].bitcast(U16).rearrange(
                                "p r u -> p (r u)"
                            ),
                        )
                pairTs[k] = pt[:].bitcast(F8).rearrange(
                    "p j (c t) -> p j t c", t=2
                )
                produced[k] = True
                yield

            def mm_sweep(g, interleave):
                chunks = GROUPS[g]
                gw = len(chunks)
                for bt in range(BT):
                    ps = ps_pool.tile([128, 2048], F32, tag="ps")
                    for j in range(2):
                        for ci, (bu, half) in enumerate(chunks):
                            nc.tensor.matmul(
                                out=ps[:, ci * 512:(ci + 1) * 512],
                                lhsT=xTw[:, j, :, bt * 128:(bt + 1) * 128],
                                rhs=pairTs[bu][
                                    :, j, :, half * 512:(half + 1) * 512
                                ],
                                start=(j == 0), stop=(j == 1),
                                perf_mode=DR,
                            )
                        for _ in range(2):
                            next(interleave, None)
                    edump = ed_pool.tile([128, 2048], BF16, tag="edump")
                    nc.scalar.activation(
                        edump[:, :gw * 512], ps[:, :gw * 512], AF.Exp,
                        scale=EXPSC,
                        accum_out=zp2d[:, bt * NG + g:bt * NG + g + 1],
                    )

            def disc_gen():
                # ---------------- disc-loss gather path ----------------
                for bt in range(BT):
                    nc.gpsimd.indirect_dma_start(
                        out=g3[:, bt, :], out_offset=None,
                        in_=ia_d[:, :],
                        in_offset=bass.IndirectOffsetOnAxis(
                            ap=toffs[:, bt:bt + 1], axis=0
                        ),
                    )
                    nc.gpsimd.indirect_dma_start(
                        out=btg3[:, bt, :], out_offset=None,
                        in_=bsh_d[:, :],
                        in_offset=bass.IndirectOffsetOnAxis(
                            ap=toffs[:, bt:bt + 1], axis=0
                        ),
                    )
                    dmp = dump_pool.tile([128, D], F32, tag="dmpf32")
                    nc.vector.scalar_tensor_tensor(
                        out=dmp[:], in0=g3[:, bt, :], scalar=1.0,
                        in1=g3[:, bt, :], op0=ALU.mult, op1=ALU.mult,
                        accum_out=ng2[:, bt:bt + 1],
                    )
                    dmp = dump_pool.tile([128, D], F32, tag="dmpf32")
                    nc.vector.scalar_tensor_tensor(
                        out=dmp[:], in0=g3[:, bt, :], scalar=1.0,
                        in1=xn3[:, bt, :], op0=ALU.mult, op1=ALU.mult,
                        accum_out=dot8[:, bt:bt + 1],
                    )
                    dmp = dump_pool.tile([128, D], F32, tag="dmpf32")
                    nc.vector.scalar_tensor_tensor(
                        out=dmp[:], in0=btg3[:, bt, :], scalar=1.0,
                        in1=btg3[:, bt, :], op0=ALU.mult, op1=ALU.mult,
                        accum_out=btn2[:, bt:bt + 1],
                    )
                    yield
                nc.vector.tensor_scalar_max(out=ng2[:], in0=ng2[:], scalar1=1e-30)
                nc.vector.tensor_scalar_max(out=btn2[:], in0=btn2[:], scalar1=1e-30)
                nc.scalar.activation(lb8[:], ng2[:], AF.Ln)
                nc.scalar.activation(s1_8[:], lb8[:], AF.Exp, scale=-0.5)
                nc.scalar.activation(lb8[:], btn2[:], AF.Ln)
                nc.scalar.activation(f8t[:], lb8[:], AF.Exp, scale=-0.5, bias=LOG_BCLIP)
                nc.vector.tensor_scalar_min(out=f8t[:], in0=f8t[:], scalar1=1.0)
                yield
                for bt in range(BT):
                    t1 = w_pool.tile([128, D], F32, tag="wk")
                    nc.vector.scalar_tensor_tensor(
                        out=t1[:], in0=g3[:, bt, :], scalar=s1_8[:, bt:bt + 1],
                        in1=xn3[:, bt, :], op0=ALU.mult, op1=ALU.subtract,
                    )
                    t2 = w_pool.tile([128, D], F32, tag="wk")
                    dmp = dump_pool.tile([128, D], F32, tag="dmpf32")
                    nc.vector.scalar_tensor_tensor(
                        out=t2[:], in0=btg3[:, bt, :], scalar=f8t[:, bt:bt + 1],
                        in1=t1[:], op0=ALU.mult, op1=ALU.add,
                    )
                    nc.vector.scalar_tensor_tensor(
                        out=dmp[:], in0=t2[:], scalar=1.0,
                        in1=t2[:], op0=ALU.mult, op1=ALU.mult,
                        accum_out=rn2[:, bt:bt + 1],
                    )
                    yield
                nc.vector.tensor_scalar_max(out=rn2[:], in0=rn2[:], scalar1=1e-30)
                nc.scalar.activation(lb8[:], rn2[:], AF.Ln)
                nc.scalar.activation(lb8[:], lb8[:], AF.Exp, scale=0.5)
                nc.vector.tensor_tensor(
                    out=payload1[:, 8:16], in0=lb8[:], in1=tmasks[:], op=ALU.mult
                )
                nc.vector.tensor_tensor(
                    out=s1_8[:], in0=dot8[:], in1=s1_8[:], op=ALU.mult
                )
                nc.vector.tensor_tensor(
                    out=payload1[:, 0:8], in0=s1_8[:], in1=tmasks[:], op=ALU.mult
                )
                nc.gpsimd.dma_start(out=ccin1[:], in_=payload1[:])
                nc.gpsimd.collective_compute(
                    "AllReduce", ALU.add,
                    replica_groups=[list(range(NCORES))],
                    ins=[ccin1[:]], outs=[ccout1[:]],
                )
                nc.gpsimd.dma_start(out=allred1[:], in_=ccout1[:])
                e1 = s_pool.tile([128, 8], F32, tag="e1")
                e2 = s_pool.tile([128, 8], F32, tag="e2")
                eref["e1"], eref["e2"] = e1, e2
                nc.scalar.activation(e1[:], allred1[:, 0:8], AF.Exp, scale=SCALE)
                nc.scalar.activation(
                    e2[:], allred1[:, 0:8], AF.Exp, scale=SCALE, bias=-SM
                )
                yield

            eref = {}

            def producer_chain():
                dg = disc_gen()
                for k in range(NBU):
                    for _ in produce(k):
                        yield
                        if k >= 1:
                            next(dg, None)
                            yield
                for _ in dg:
                    yield

            prod = producer_chain()
            for g in range(NG):
                need = {bu for bu, _ in GROUPS[g]}
                while not all(produced[bu] for bu in need):
                    next(prod)
                mm_sweep(g, prod)
            for _ in prod:
                pass

            # ---------------- reduce Z partials ----------------
            for bt in range(BT):
                nc.vector.reduce_sum(
                    out=payload2[:, bt:bt + 1],
                    in_=zp2d[:, bt * NG:(bt + 1) * NG],
                    axis=AX.X,
                )

            # ---------------- all-reduce of Z partials ----------------
            nc.gpsimd.dma_start(out=ccin2[:], in_=payload2[:])
            nc.gpsimd.collective_compute(
                "AllReduce", ALU.add,
                replica_groups=[list(range(NCORES))],
                ins=[ccin2[:]], outs=[ccout2[:]],
            )
            nc.gpsimd.dma_start(out=allred2[:], in_=ccout2[:])

            # ---------------- final loss math (identical on all cores) -------
            zsum = allred2[:, 0:8]
            st8 = allred1[:, 0:8]
            rn8 = allred1[:, 8:16]
            zc = s_pool.tile([128, 8], F32, tag="zc")
            lnz = s_pool.tile([128, 8], F32, tag="lnz")
            nll = s_pool.tile([128, 8], F32, tag="nll")
            nc.vector.tensor_scalar_add(
                out=zc[:], in0=zsum, scalar1=-NPAD_TOTAL
            )
            e1, e2 = eref["e1"], eref["e2"]
            nc.vector.tensor_tensor(out=zc[:], in0=zc[:], in1=e1[:], op=ALU.subtract)
            nc.vector.tensor_tensor(out=zc[:], in0=zc[:], in1=e2[:], op=ALU.add)
            nc.scalar.activation(lnz[:], zc[:], AF.Ln)
            nc.vector.scalar_tensor_tensor(
                out=nll[:], in0=st8, scalar=-SCALE, in1=lnz[:],
                op0=ALU.mult, op1=ALU.add,
            )
            nc.vector.tensor_scalar_add(out=nll[:], in0=nll[:], scalar1=SM)
            red2 = s_pool.tile([128, 2], F32, tag="red2")
            nc.vector.reduce_sum(out=red2[:, 0:1], in_=nll[:], axis=AX.X)
            nc.vector.reduce_sum(out=red2[:, 1:2], in_=rn8, axis=AX.X)
            fin_ps = ps_pool.tile([128, 2048], F32, tag="ps")
            nc.tensor.matmul(
                out=fin_ps[0:1, 0:2], lhsT=ones[:], rhs=red2[:],
                start=True, stop=True,
            )
            fin = s_pool.tile([1, 2], F32, tag="fin")
            nc.vector.tensor_copy(out=fin[:], in_=fin_ps[0:1, 0:2])
            p_t = s_pool.tile([1, 1], F32, tag="p_t")
            nc.scalar.activation(p_t[:], fin[:, 0:1], AF.Exp, scale=-1.0 / B)
            q_t = s_pool.tile([1, 1], F32, tag="q_t")
            nc.vector.tensor_scalar(
                out=q_t[:], in0=p_t[:], scalar1=-1.0, scalar2=1.0,
                op0=ALU.mult, op1=ALU.add,
            )
            nc.vector.tensor_tensor(out=q_t[:], in0=q_t[:], in1=q_t[:], op=ALU.mult)
            lgp = s_pool.tile([1, 1], F32, tag="lgp")
            nc.vector.tensor_scalar_mul(out=lgp[:], in0=fin[:, 0:1], scalar1=1.0 / B)
            nc.vector.tensor_tensor(out=q_t[:], in0=q_t[:], in1=lgp[:], op=ALU.mult)
            rterm = s_pool.tile([1, 1], F32, tag="rterm")
            nc.vector.tensor_scalar_mul(
                out=rterm[:], in0=fin[:, 1:2], scalar1=LAMBDA / B
            )
            nc.vector.tensor_tensor(
                out=q_t[:], in0=q_t[:], in1=rterm[:], op=ALU.add
            )
            nc.gpsimd.dma_start(out=out_d[:], in_=q_t[:])

    n = split_multi_waits(nc)
    print(f"split_multi_waits: inserted {n} wait-nops")
    return nc


_NC_CACHE = {}


def _get_nc():
    if "nc" not in _NC_CACHE:
        _NC_CACHE["nc"] = build_bass()
    return _NC_CACHE["nc"]


def make_in_maps(x, target, id_agent, b):
    x = np.ascontiguousarray(np.asarray(x, dtype=np.float32))
    target = np.asarray(target).astype(np.int64)
    id_agent = np.asarray(id_agent, dtype=np.float32)
    b = np.asarray(b, dtype=np.float32)

    in_maps = []
    for k in range(NCORES):
        lo = k * CPER
        ia_k = np.zeros((CSH, D), dtype=np.float32)
        ia_k[:CPER] = id_agent[lo:lo + CPER]
        b_k = np.zeros((CSH, D), dtype=np.float32)
        b_k[:CPER] = b[lo:lo + CPER]
        tloc = np.clip(target - lo, 0, CPER - 1).astype(np.int32)
        owned = ((target >= lo) & (target < lo + CPER)).astype(np.float32)
        toff_k = np.ascontiguousarray(tloc.reshape(BT, 128).T)
        tmask_k = np.ascontiguousarray(owned.reshape(BT, 128).T)
        in_maps.append(
            {
                "x": x,
                "ia": ia_k,
                "bsh": b_k,
                "toff": toff_k,
                "tmask": tmask_k,
            }
        )
    return in_maps


def run(inputs, trace=False, **kw):
    nc = _get_nc()
    in_maps = make_in_maps(**inputs)
    res = run_bass_kernel_spmd(
        nc, in_maps, core_ids=list(range(NCORES)), trace=trace, **kw
    )
    return res


def kernel(x, target, id_agent, b):
    res = run({"x": x, "target": target, "id_agent": id_agent, "b": b})
    return np.asarray(res.results[0]["out"], dtype=np.float32)
